# revision 15
# baseline (speedup 1.0000x reference)
"""Trainium2 Bass kernel for ComplexProjection:
    out[b,r,p] = |sum_s complex(x_real,x_imag)[b,r,s] * projection[r,s,p]|

Data-parallel over B across 8 NeuronCores.  Per core (B/8 = 4096):

  x shipped as [r, s, {re, im}, b] fp8 e3m4   (16.8 MB)
  w as [s, r, p] fp16 (stationary, mixed-dtype matmul)
  device computes ssq/4 = (re/2)^2 + (im/2)^2 in fp16, stores fp8 e3m4
  [p, r, b] via SWDGE cast-during-DMA (8.4 MB); host does 2*sqrt(o).

Per chunk (1024 b-columns) one [128, 2048] PSUM tile (4 banks, x2 bufs)
holds re | im halves, filled by 4 matmuls (N=512, W stationary per r).
Epilogue rotates two patterns so ACT/DVE/GPSIMD land ~equal busy time
(fp32 PSUM reads are 1x on ACT/DVE and are the bottleneck; fp16 SBUF
keeps DVE tensor ops in 2x mode; GPSIMD cannot read PSUM):
  P1: ACT squares the whole 2048 tile (scale 0.5) -> sq fp16;
      DVE (or GP on a few chunks) adds halves -> out fp16
  P2: ACT squares im half; DVE copies re half with scale 0.5 (cast);
      GP squares the fp16 copy; DVE adds -> out fp16
"""

import os

import numpy as np

B, R, S, P = 32768, 16, 128, 128
NCORES = 8
BC = B // NCORES  # 4096 particles per core
CH = 1024         # output chunk (psum tile = [128, 2*CH] fp32 = 4 banks)
MMN = 512         # matmul moving dim (one psum bank)
NCH = BC // CH    # 4 chunks per r

NR_ST = int(os.environ.get("KNRST", "2"))    # r's per output store
N1 = int(os.environ.get("KN1", "5"))         # P1 chunks per 16
GADD = int(os.environ.get("KGADD", "0"))     # GP adds per 16 (on P1 chunks)
GPOP = os.environ.get("KGPOP", "mult")       # pow | mult  (GP square op)
P2SQ = os.environ.get("KP2SQ", "gp")         # gp | dvepow | dvett
KFILL = int(os.environ.get("KFILL", "0"))    # filler ldweights per chunk
KDUP = int(os.environ.get("KDUP", "100"))    # matmul duplication percent
                                             # (100 = none, 200 = issue every
                                             #  matmul twice).  Redundant PE
                                             #  work keeps the HAM activity
                                             #  monitor busy so the 2.4 GHz
                                             #  clock gate stays open; the
                                             #  second issue overwrites the
                                             #  same PSUM bank (start=True).
KST = os.environ.get("KST", "swdge8")        # swdge8 | hw16
XBUFS = int(os.environ.get("KXBUFS", "3"))
OBUFS = int(os.environ.get("KOBUFS", "3"))

_prog_cache = {}


def _build(nc, tile, mybir):
    f32 = mybir.dt.float32
    f16 = mybir.dt.float16
    fp8 = mybir.dt.float8e3
    odt = fp8 if KST == "swdge8" else f16

    x = nc.dram_tensor("x", [R, S, 2, BC], fp8, kind="ExternalInput")
    w = nc.dram_tensor("w", [S, R, P], f16, kind="ExternalInput")
    # [p, r, b]: per-partition contiguous r-blocks for big SWDGE stores
    o = nc.dram_tensor("o", [P, R, BC], odt, kind="ExternalOutput")
    x_ap, w_ap, o_ap = x.ap(), w.ap(), o.ap()

    # per-16 chunk schedule: (pattern, add_engine)
    sched = []
    p1_pos = set(round(i * 16 / max(N1, 1)) for i in range(N1))
    g_left = GADD
    for i in range(16):
        if i in p1_pos:
            if g_left > 0:
                sched.append(("P1", "G"))
                g_left -= 1
            else:
                sched.append(("P1", "V"))
        else:
            sched.append(("P2", "V"))

    with tile.TileContext(nc) as tc:
        with (
            tc.tile_pool(name="wp", bufs=1) as wp,
            tc.tile_pool(name="xp", bufs=XBUFS) as xp,
            tc.tile_pool(name="op", bufs=OBUFS) as op,
            tc.tile_pool(name="sq", bufs=6) as sqp,
            tc.tile_pool(name="cp", bufs=6) as cpp,
            tc.tile_pool(name="ps", bufs=2, space="PSUM") as psp,
        ):
            w_sb = wp.tile([S, R, P], f16, tag="w")
            nc.sync.dma_start(w_sb[:], w_ap[:])

            ci = 0
            out_sb = None
            for r in range(R):
                wr = w_sb[:, r, :]
                x_sb = xp.tile([S, 2, BC], fp8, tag="x")
                if r == 0:
                    # split the very first slab so the first matmuls
                    # start as early as possible
                    q = BC // 8
                    for h in range(8):
                        nc.sync.dma_start(
                            x_sb[:, :, h * q:(h + 1) * q],
                            x_ap[r, :, :, h * q:(h + 1) * q])
                else:
                    nc.sync.dma_start(x_sb[:], x_ap[r, :, :, :])
                j = r % NR_ST
                if j == 0:
                    out_sb = op.tile([P, NR_ST, BC], f16, tag="o")
                for cc in range(NCH):
                    osl = slice(cc * CH, (cc + 1) * CH)
                    pat, add_e = sched[ci % 16]
                    ci += 1
                    ps = psp.tile([P, 2 * CH], f32, tag="ps")
                    for comp in range(2):
                        for m in range(CH // MMN):
                            msl = slice(comp * CH + m * MMN,
                                        comp * CH + (m + 1) * MMN)
                            xsl = slice(cc * CH + m * MMN,
                                        cc * CH + (m + 1) * MMN)
                            self_dup = (ci * 2 + comp) * (KDUP - 100)
                            reps = 1 + (self_dup % 100 + KDUP - 100) // 100
                            for _ in range(max(reps, 1)):
                                nc.tensor.matmul(ps[:, msl], wr,
                                                 x_sb[:, comp, xsl],
                                                 start=True, stop=True)
                    for _ in range(KFILL):
                        nc.tensor.ldweights(wr)
                    if pat == "P1":
                        sq = sqp.tile([P, 2 * CH], f16, tag="sq")
                        nc.scalar.activation(
                            sq[:], ps[:],
                            mybir.ActivationFunctionType.Square, scale=0.5)
                        eng = nc.vector if add_e == "V" else nc.gpsimd
                        eng.tensor_add(out_sb[:, j, osl],
                                       sq[:, :CH], sq[:, CH:])
                    else:
                        sq_i = sqp.tile([P, CH], f16, tag="sqi")
                        nc.scalar.activation(
                            sq_i[:], ps[:, CH:],
                            mybir.ActivationFunctionType.Square, scale=0.5)
                        sq_r = cpp.tile([P, CH], f16, tag="sqr")
                        if P2SQ == "dvepow":
                            # fused (ps*0.5)^2 straight out of PSUM on DVE
                            nc.vector.tensor_scalar(
                                sq_r[:], ps[:, :CH], 0.5, 2.0,
                                mybir.AluOpType.mult, mybir.AluOpType.pow)
                        elif P2SQ == "dvett":
                            cp_r = cpp.tile([P, CH], f16, tag="cpr")
                            nc.vector.tensor_scalar_mul(cp_r[:], ps[:, :CH], 0.5)
                            nc.vector.tensor_mul(sq_r[:], cp_r[:], cp_r[:])
                        else:
                            cp_r = cpp.tile([P, CH], f16, tag="cpr")
                            nc.vector.tensor_scalar_mul(cp_r[:], ps[:, :CH], 0.5)
                            if GPOP == "pow":
                                nc.gpsimd.tensor_scalar(
                                    sq_r[:], cp_r[:], 2.0, None,
                                    mybir.AluOpType.pow)
                            else:
                                nc.gpsimd.tensor_mul(sq_r[:], cp_r[:], cp_r[:])
                        nc.vector.tensor_add(out_sb[:, j, osl],
                                             sq_r[:], sq_i[:])
                if j == NR_ST - 1:
                    r0 = r - (NR_ST - 1)
                    dst = o_ap[:, r0:r0 + NR_ST, :]
                    if KST == "swdge8":
                        nc.gpsimd.dma_start(dst, out_sb[:])
                    else:
                        nc.scalar.dma_start(dst, out_sb[:])


def _build_program():
    key = (N1, GADD, GPOP, KST, NR_ST, XBUFS, OBUFS, P2SQ, KFILL, KDUP)
    if key in _prog_cache:
        return _prog_cache[key]

    import concourse.tile as tile
    from concourse import bacc, mybir

    nc = bacc.Bacc("TRN2", target_bir_lowering=False, debug=False,
                   num_devices=NCORES)
    _build(nc, tile, mybir)
    nc.compile()
    _prog_cache[key] = nc
    return nc


LAST_RESULT = None


def kernel(x_real, x_imag, projection):
    global LAST_RESULT
    import ml_dtypes
    from concourse.bass_utils import run_bass_kernel_spmd

    nc = _build_program()

    w = np.ascontiguousarray(
        np.asarray(projection, dtype=np.float32).transpose(1, 0, 2)
    ).astype(np.float16)

    # x: (B, R, S) re/im fp32 -> [R, S, 2, B], sliced per core on b
    xt = np.empty((R, S, 2, B), dtype=ml_dtypes.float8_e3m4)
    xt[:, :, 0, :] = np.asarray(x_real, dtype=np.float32).transpose(1, 2, 0)
    xt[:, :, 1, :] = np.asarray(x_imag, dtype=np.float32).transpose(1, 2, 0)

    in_maps = []
    for c in range(NCORES):
        sl = slice(c * BC, (c + 1) * BC)
        in_maps.append({"x": np.ascontiguousarray(xt[:, :, :, sl]), "w": w})

    res = run_bass_kernel_spmd(nc, in_maps, core_ids=list(range(NCORES)))
    LAST_RESULT = res
    out = np.empty((B, R, P), dtype=np.float32)
    for c in range(NCORES):
        ssq4 = res.results[c]["o"].astype(np.float32)  # [P, R, BC] of ssq/4
        out[c * BC:(c + 1) * BC] = 2.0 * np.sqrt(ssq4).transpose(2, 1, 0)
    return out


# revision 18
# speedup vs baseline: 1.1418x; 1.1418x over previous
"""Trainium2 Bass kernel for ComplexProjection:
    out[b,r,p] = |sum_s complex(x_real,x_imag)[b,r,s] * projection[r,s,p]|

Data-parallel over B across 8 NeuronCores.  Per core (B/8 = 4096):

  x shipped as [r, s, {re, im}, b] fp8 e3m4   (16.8 MB)
  w as [s, r, p] fp16 (stationary, mixed-dtype matmul)
  device computes ssq/4 = (re/2)^2 + (im/2)^2 in fp16, stores fp8 e3m4
  [p, r, b] via SWDGE cast-during-DMA (8.4 MB); host does 2*sqrt(o).

Per chunk (1024 b-columns) one [128, 2048] PSUM tile (4 banks, x2 bufs)
holds re | im halves, filled by 4 matmuls (N=512, W stationary per r).
Epilogue rotates two patterns so ACT/DVE/GPSIMD land ~equal busy time
(fp32 PSUM reads are 1x on ACT/DVE and are the bottleneck; fp16 SBUF
keeps DVE tensor ops in 2x mode; GPSIMD cannot read PSUM):
  P1: ACT squares the whole 2048 tile (scale 0.5) -> sq fp16;
      DVE (or GP on a few chunks) adds halves -> out fp16
  P2: ACT squares im half; DVE copies re half with scale 0.5 (cast);
      GP squares the fp16 copy; DVE adds -> out fp16
"""

import os

import numpy as np

B, R, S, P = 32768, 16, 128, 128
NCORES = 8
BC = B // NCORES  # 4096 particles per core
CH = 1024         # output chunk (psum tile = [128, 2*CH] fp32 = 4 banks)
MMN = 512         # matmul moving dim (one psum bank)
NCH = BC // CH    # 4 chunks per r

NR_ST = int(os.environ.get("KNRST", "2"))    # r's per output store
N1 = int(os.environ.get("KN1", "5"))         # P1 chunks per 16
GADD = int(os.environ.get("KGADD", "0"))     # GP adds per 16 (on P1 chunks)
GPOP = os.environ.get("KGPOP", "mult")       # pow | mult  (GP square op)
P2SQ = os.environ.get("KP2SQ", "gp")         # gp | dvepow | dvett
KFILL = int(os.environ.get("KFILL", "0"))    # filler ldweights per chunk
KDUP = int(os.environ.get("KDUP", "100"))    # matmul duplication percent
                                             # (100 = none, 200 = issue every
                                             #  matmul twice).  Redundant PE
                                             #  work keeps the HAM activity
                                             #  monitor busy so the 2.4 GHz
                                             #  clock gate stays open; the
                                             #  second issue overwrites the
                                             #  same PSUM bank (start=True).
KDUPN = int(os.environ.get("KDUPN", "0"))    # apply KDUP only to the first
                                             # N chunks (0 = all chunks):
                                             # early redundancy pushes the
                                             # HAM into the warm basin, then
                                             # high duty sustains it.
KST = os.environ.get("KST", "swdge8")        # swdge8 | hw16
XBUFS = int(os.environ.get("KXBUFS", "3"))
OBUFS = int(os.environ.get("KOBUFS", "3"))

_prog_cache = {}


def _build(nc, tile, mybir):
    f32 = mybir.dt.float32
    f16 = mybir.dt.float16
    fp8 = mybir.dt.float8e3
    odt = fp8 if KST == "swdge8" else f16

    x = nc.dram_tensor("x", [R, S, 2, BC], fp8, kind="ExternalInput")
    w = nc.dram_tensor("w", [S, R, P], f16, kind="ExternalInput")
    # [p, r, b]: per-partition contiguous r-blocks for big SWDGE stores
    o = nc.dram_tensor("o", [P, R, BC], odt, kind="ExternalOutput")
    x_ap, w_ap, o_ap = x.ap(), w.ap(), o.ap()

    # per-16 chunk schedule: (pattern, add_engine)
    sched = []
    p1_pos = set(round(i * 16 / max(N1, 1)) for i in range(N1))
    g_left = GADD
    for i in range(16):
        if i in p1_pos:
            if g_left > 0:
                sched.append(("P1", "G"))
                g_left -= 1
            else:
                sched.append(("P1", "V"))
        else:
            sched.append(("P2", "V"))

    with tile.TileContext(nc) as tc:
        with (
            tc.tile_pool(name="wp", bufs=1) as wp,
            tc.tile_pool(name="xp", bufs=XBUFS) as xp,
            tc.tile_pool(name="op", bufs=OBUFS) as op,
            tc.tile_pool(name="sq", bufs=6) as sqp,
            tc.tile_pool(name="cp", bufs=6) as cpp,
            tc.tile_pool(name="ps", bufs=2, space="PSUM") as psp,
        ):
            w_sb = wp.tile([S, R, P], f16, tag="w")
            nc.sync.dma_start(w_sb[:], w_ap[:])

            ci = 0
            out_sb = None
            for r in range(R):
                wr = w_sb[:, r, :]
                x_sb = xp.tile([S, 2, BC], fp8, tag="x")
                if r == 0:
                    # split the very first slab so the first matmuls
                    # start as early as possible
                    q = BC // 8
                    for h in range(8):
                        nc.sync.dma_start(
                            x_sb[:, :, h * q:(h + 1) * q],
                            x_ap[r, :, :, h * q:(h + 1) * q])
                else:
                    nc.sync.dma_start(x_sb[:], x_ap[r, :, :, :])
                j = r % NR_ST
                if j == 0:
                    out_sb = op.tile([P, NR_ST, BC], f16, tag="o")
                for cc in range(NCH):
                    osl = slice(cc * CH, (cc + 1) * CH)
                    pat, add_e = sched[ci % 16]
                    ci += 1
                    ps = psp.tile([P, 2 * CH], f32, tag="ps")
                    for comp in range(2):
                        for m in range(CH // MMN):
                            msl = slice(comp * CH + m * MMN,
                                        comp * CH + (m + 1) * MMN)
                            xsl = slice(cc * CH + m * MMN,
                                        cc * CH + (m + 1) * MMN)
                            self_dup = (ci * 2 + comp) * (KDUP - 100)
                            reps = 1 + (self_dup % 100 + KDUP - 100) // 100
                            if KDUPN and ci >= KDUPN:
                                reps = 1
                            for _ in range(max(reps, 1)):
                                nc.tensor.matmul(ps[:, msl], wr,
                                                 x_sb[:, comp, xsl],
                                                 start=True, stop=True)
                    for _ in range(KFILL):
                        nc.tensor.ldweights(wr)
                    if pat == "P1":
                        sq = sqp.tile([P, 2 * CH], f16, tag="sq")
                        nc.scalar.activation(
                            sq[:], ps[:],
                            mybir.ActivationFunctionType.Square, scale=0.5)
                        eng = nc.vector if add_e == "V" else nc.gpsimd
                        eng.tensor_add(out_sb[:, j, osl],
                                       sq[:, :CH], sq[:, CH:])
                    else:
                        sq_i = sqp.tile([P, CH], f16, tag="sqi")
                        nc.scalar.activation(
                            sq_i[:], ps[:, CH:],
                            mybir.ActivationFunctionType.Square, scale=0.5)
                        sq_r = cpp.tile([P, CH], f16, tag="sqr")
                        if P2SQ == "dvepow":
                            # fused (ps*0.5)^2 straight out of PSUM on DVE
                            nc.vector.tensor_scalar(
                                sq_r[:], ps[:, :CH], 0.5, 2.0,
                                mybir.AluOpType.mult, mybir.AluOpType.pow)
                        elif P2SQ == "dvett":
                            cp_r = cpp.tile([P, CH], f16, tag="cpr")
                            nc.vector.tensor_scalar_mul(cp_r[:], ps[:, :CH], 0.5)
                            nc.vector.tensor_mul(sq_r[:], cp_r[:], cp_r[:])
                        else:
                            cp_r = cpp.tile([P, CH], f16, tag="cpr")
                            nc.vector.tensor_scalar_mul(cp_r[:], ps[:, :CH], 0.5)
                            if GPOP == "pow":
                                nc.gpsimd.tensor_scalar(
                                    sq_r[:], cp_r[:], 2.0, None,
                                    mybir.AluOpType.pow)
                            else:
                                nc.gpsimd.tensor_mul(sq_r[:], cp_r[:], cp_r[:])
                        nc.vector.tensor_add(out_sb[:, j, osl],
                                             sq_r[:], sq_i[:])
                if j == NR_ST - 1:
                    r0 = r - (NR_ST - 1)
                    dst = o_ap[:, r0:r0 + NR_ST, :]
                    if KST == "swdge8":
                        nc.gpsimd.dma_start(dst, out_sb[:])
                    else:
                        nc.scalar.dma_start(dst, out_sb[:])


def _build_program():
    key = (N1, GADD, GPOP, KST, NR_ST, XBUFS, OBUFS, P2SQ, KFILL, KDUP,
           KDUPN)
    if key in _prog_cache:
        return _prog_cache[key]

    import concourse.tile as tile
    from concourse import bacc, mybir

    nc = bacc.Bacc("TRN2", target_bir_lowering=False, debug=False,
                   num_devices=NCORES)
    _build(nc, tile, mybir)
    nc.compile()
    _prog_cache[key] = nc
    return nc


LAST_RESULT = None


def kernel(x_real, x_imag, projection):
    global LAST_RESULT
    import ml_dtypes
    from concourse.bass_utils import run_bass_kernel_spmd

    nc = _build_program()

    w = np.ascontiguousarray(
        np.asarray(projection, dtype=np.float32).transpose(1, 0, 2)
    ).astype(np.float16)

    # x: (B, R, S) re/im fp32 -> [R, S, 2, B], sliced per core on b
    xt = np.empty((R, S, 2, B), dtype=ml_dtypes.float8_e3m4)
    xt[:, :, 0, :] = np.asarray(x_real, dtype=np.float32).transpose(1, 2, 0)
    xt[:, :, 1, :] = np.asarray(x_imag, dtype=np.float32).transpose(1, 2, 0)

    in_maps = []
    for c in range(NCORES):
        sl = slice(c * BC, (c + 1) * BC)
        in_maps.append({"x": np.ascontiguousarray(xt[:, :, :, sl]), "w": w})

    res = run_bass_kernel_spmd(nc, in_maps, core_ids=list(range(NCORES)))
    LAST_RESULT = res
    out = np.empty((B, R, P), dtype=np.float32)
    for c in range(NCORES):
        ssq4 = res.results[c]["o"].astype(np.float32)  # [P, R, BC] of ssq/4
        out[c * BC:(c + 1) * BC] = 2.0 * np.sqrt(ssq4).transpose(2, 1, 0)
    return out


# revision 24
# speedup vs baseline: 1.3909x; 1.2182x over previous
"""Trainium2 Bass kernel for ComplexProjection:
    out[b,r,p] = |sum_s complex(x_real,x_imag)[b,r,s] * projection[r,s,p]|

Data-parallel over B across 8 NeuronCores.  Per core (B/8 = 4096):

  x shipped as [r, s, {re, im}, b] fp8 e3m4   (16.8 MB)
  w as [s, r, p] fp16 (stationary, mixed-dtype matmul)
  device computes ssq/4 = (re/2)^2 + (im/2)^2 in fp16, stores fp8 e3m4
  [p, r, b] via SWDGE cast-during-DMA (8.4 MB); host does 2*sqrt(o).

Per chunk (1024 b-columns) one [128, 2048] PSUM tile (4 banks, x2 bufs)
holds re | im halves, filled by 4 matmuls (N=512, W stationary per r).
Epilogue rotates two patterns so ACT/DVE/GPSIMD land ~equal busy time
(fp32 PSUM reads are 1x on ACT/DVE and are the bottleneck; fp16 SBUF
keeps DVE tensor ops in 2x mode; GPSIMD cannot read PSUM):
  P1: ACT squares the whole 2048 tile (scale 0.5) -> sq fp16;
      DVE (or GP on a few chunks) adds halves -> out fp16
  P2: ACT squares im half; DVE copies re half with scale 0.5 (cast);
      GP squares the fp16 copy; DVE adds -> out fp16
"""

import os

import numpy as np

B, R, S, P = 32768, 16, 128, 128
NCORES = 8
BC = B // NCORES  # 4096 particles per core
CH = 1024         # output chunk (psum tile = [128, 2*CH] fp32 = 4 banks)
MMN = 512         # matmul moving dim (one psum bank)
NCH = BC // CH    # 4 chunks per r

NR_ST = int(os.environ.get("KNRST", "2"))    # r's per output store
N1 = int(os.environ.get("KN1", "5"))         # P1 chunks per 16
GADD = int(os.environ.get("KGADD", "0"))     # GP adds per 16 (on P1 chunks)
GPOP = os.environ.get("KGPOP", "mult")       # pow | mult  (GP square op)
P2SQ = os.environ.get("KP2SQ", "gp")         # gp | dvepow | dvett
KFILL = int(os.environ.get("KFILL", "0"))    # filler ldweights per chunk
KDUP = int(os.environ.get("KDUP", "100"))    # matmul duplication percent
                                             # (100 = none, 200 = issue every
                                             #  matmul twice).  Redundant PE
                                             #  work keeps the HAM activity
                                             #  monitor busy so the 2.4 GHz
                                             #  clock gate stays open; the
                                             #  second issue overwrites the
                                             #  same PSUM bank (start=True).
KDUPN = int(os.environ.get("KDUPN", "0"))    # apply KDUP only to the first
                                             # N chunks (0 = all chunks):
                                             # early redundancy pushes the
                                             # HAM into the warm basin, then
                                             # high duty sustains it.
KST = os.environ.get("KST", "swdge8")        # swdge8 | hw16
KADL = int(os.environ.get("KADL", "1"))      # chunks of add-emission delay
XBUFS = int(os.environ.get("KXBUFS", "3"))
OBUFS = int(os.environ.get("KOBUFS", "3"))

_prog_cache = {}


def _build(nc, tile, mybir):
    f32 = mybir.dt.float32
    f16 = mybir.dt.float16
    fp8 = mybir.dt.float8e3
    odt = fp8 if KST == "swdge8" else f16

    x = nc.dram_tensor("x", [R, S, 2, BC], fp8, kind="ExternalInput")
    w = nc.dram_tensor("w", [S, R, P], f16, kind="ExternalInput")
    # [p, r, b]: per-partition contiguous r-blocks for big SWDGE stores
    o = nc.dram_tensor("o", [P, R, BC], odt, kind="ExternalOutput")
    x_ap, w_ap, o_ap = x.ap(), w.ap(), o.ap()

    # per-16 chunk schedule: (pattern, add_engine)
    sched = []
    p1_pos = set(round(i * 16 / max(N1, 1)) for i in range(N1))
    g_left = GADD
    for i in range(16):
        if i in p1_pos:
            if g_left > 0:
                sched.append(("P1", "G"))
                g_left -= 1
            else:
                sched.append(("P1", "V"))
        else:
            sched.append(("P2", "V"))

    with tile.TileContext(nc) as tc:
        with (
            tc.tile_pool(name="wp", bufs=1) as wp,
            tc.tile_pool(name="xp", bufs=XBUFS) as xp,
            tc.tile_pool(name="op", bufs=OBUFS) as op,
            tc.tile_pool(name="sq", bufs=8) as sqp,
            tc.tile_pool(name="cp", bufs=8) as cpp,
            tc.tile_pool(name="ps", bufs=2, space="PSUM") as psp,
        ):
            w_sb = wp.tile([S, R, P], f16, tag="w")
            nc.sync.dma_start(w_sb[:], w_ap[:])

            # adds are emitted KADL chunks late so the DVE queue never
            # head-of-line blocks on the GP/ACT square feeding an add
            # while later casts are already runnable
            pending = []           # (emit_fn, group_id)
            adds_left = {}         # group_id -> adds not yet emitted
            store_fn = {}          # group_id -> store emitter

            def emit_one_pending():
                fn, gid = pending.pop(0)
                fn()
                adds_left[gid] -= 1
                if adds_left[gid] == 0 and gid in store_fn:
                    store_fn.pop(gid)()

            ci = 0
            out_sb = None
            for r in range(R):
                wr = w_sb[:, r, :]
                x_sb = xp.tile([S, 2, BC], fp8, tag="x")
                if r == 0:
                    # split the very first slab so the first matmuls
                    # start as early as possible
                    q = BC // 8
                    for h in range(8):
                        nc.sync.dma_start(
                            x_sb[:, :, h * q:(h + 1) * q],
                            x_ap[r, :, :, h * q:(h + 1) * q])
                else:
                    nc.sync.dma_start(x_sb[:], x_ap[r, :, :, :])
                j = r % NR_ST
                gid = r // NR_ST
                if j == 0:
                    out_sb = op.tile([P, NR_ST, BC], f16, tag="o")
                    adds_left[gid] = NR_ST * NCH
                for cc in range(NCH):
                    osl = slice(cc * CH, (cc + 1) * CH)
                    pat, add_e = sched[ci % 16]
                    ci += 1
                    ps = psp.tile([P, 2 * CH], f32, tag="ps")
                    for comp in range(2):
                        for m in range(CH // MMN):
                            msl = slice(comp * CH + m * MMN,
                                        comp * CH + (m + 1) * MMN)
                            xsl = slice(cc * CH + m * MMN,
                                        cc * CH + (m + 1) * MMN)
                            self_dup = (ci * 2 + comp) * (KDUP - 100)
                            reps = 1 + (self_dup % 100 + KDUP - 100) // 100
                            if KDUPN and ci >= KDUPN:
                                reps = 1
                            for _ in range(max(reps, 1)):
                                nc.tensor.matmul(ps[:, msl], wr,
                                                 x_sb[:, comp, xsl],
                                                 start=True, stop=True)
                    for _ in range(KFILL):
                        nc.tensor.ldweights(wr)
                    if pat == "P1":
                        sq = sqp.tile([P, 2 * CH], f16, tag="sq")
                        nc.scalar.activation(
                            sq[:], ps[:],
                            mybir.ActivationFunctionType.Square, scale=0.5)
                        eng = nc.vector if add_e == "V" else nc.gpsimd

                        def add_fn(eng=eng, osb=out_sb, j=j, osl=osl, sq=sq):
                            eng.tensor_add(osb[:, j, osl],
                                           sq[:, :CH], sq[:, CH:])
                    else:
                        sq_i = sqp.tile([P, CH], f16, tag="sqi")
                        nc.scalar.activation(
                            sq_i[:], ps[:, CH:],
                            mybir.ActivationFunctionType.Square, scale=0.5)
                        sq_r = cpp.tile([P, CH], f16, tag="sqr")
                        if P2SQ == "dvett":
                            cp_r = cpp.tile([P, CH], f16, tag="cpr")
                            nc.vector.tensor_scalar_mul(cp_r[:], ps[:, :CH], 0.5)
                            nc.vector.tensor_mul(sq_r[:], cp_r[:], cp_r[:])
                        else:
                            cp_r = cpp.tile([P, CH], f16, tag="cpr")
                            nc.vector.tensor_scalar_mul(cp_r[:], ps[:, :CH], 0.5)
                            nc.gpsimd.tensor_mul(sq_r[:], cp_r[:], cp_r[:])

                        def add_fn(osb=out_sb, j=j, osl=osl, sq_r=sq_r,
                                   sq_i=sq_i):
                            nc.vector.tensor_add(osb[:, j, osl],
                                                 sq_r[:], sq_i[:])
                    pending.append((add_fn, gid))
                    if len(pending) > KADL:
                        emit_one_pending()
                if j == NR_ST - 1:
                    r0 = r - (NR_ST - 1)
                    dst = o_ap[:, r0:r0 + NR_ST, :]
                    if KST == "swdge8":
                        def st_fn(dst=dst, osb=out_sb):
                            nc.gpsimd.dma_start(dst, osb[:])
                    else:
                        def st_fn(dst=dst, osb=out_sb):
                            nc.scalar.dma_start(dst, osb[:])
                    if adds_left[gid] == 0:
                        st_fn()
                    else:
                        store_fn[gid] = st_fn
            while pending:
                emit_one_pending()


def _build_program():
    key = (N1, GADD, GPOP, KST, NR_ST, XBUFS, OBUFS, P2SQ, KFILL, KDUP,
           KDUPN, KADL)
    if key in _prog_cache:
        return _prog_cache[key]

    import concourse.tile as tile
    from concourse import bacc, mybir

    nc = bacc.Bacc("TRN2", target_bir_lowering=False, debug=False,
                   num_devices=NCORES)
    _build(nc, tile, mybir)
    nc.compile()
    _prog_cache[key] = nc
    return nc


LAST_RESULT = None


def kernel(x_real, x_imag, projection):
    global LAST_RESULT
    import ml_dtypes
    from concourse.bass_utils import run_bass_kernel_spmd

    nc = _build_program()

    w = np.ascontiguousarray(
        np.asarray(projection, dtype=np.float32).transpose(1, 0, 2)
    ).astype(np.float16)

    # x: (B, R, S) re/im fp32 -> [R, S, 2, B], sliced per core on b
    xt = np.empty((R, S, 2, B), dtype=ml_dtypes.float8_e3m4)
    xt[:, :, 0, :] = np.asarray(x_real, dtype=np.float32).transpose(1, 2, 0)
    xt[:, :, 1, :] = np.asarray(x_imag, dtype=np.float32).transpose(1, 2, 0)

    in_maps = []
    for c in range(NCORES):
        sl = slice(c * BC, (c + 1) * BC)
        in_maps.append({"x": np.ascontiguousarray(xt[:, :, :, sl]), "w": w})

    res = run_bass_kernel_spmd(nc, in_maps, core_ids=list(range(NCORES)))
    LAST_RESULT = res
    out = np.empty((B, R, P), dtype=np.float32)
    for c in range(NCORES):
        ssq4 = res.results[c]["o"].astype(np.float32)  # [P, R, BC] of ssq/4
        out[c * BC:(c + 1) * BC] = 2.0 * np.sqrt(ssq4).transpose(2, 1, 0)
    return out
